# revision 1
# baseline (speedup 1.0000x reference)
"""Trainium2 Bass kernel for nn_ContinualForecaster (scatter_memory).

Strategy: data-parallel over batch (B=8 -> 8 NeuronCores, one batch element
per core). The T=256 sequential state recurrence on M,S [128,128] is
reformulated as a chunked parallel scan (2 chunks of L=128):

  err_t = M k_t - v_t ; S = et*S - th*err_t k_t^T ; M = (1-al)*M + S

is linear in (M, S) given the matvecs z_t = M_{t-1} k_t. Within a chunk the
unknown update vectors w_t = -th_t(z_t - v_t) satisfy a strictly-triangular
linear system W = (I-N)^{-1} R whose coefficients come from cumulative
products of (et, 1-al) (computed stably in log space) and the key Gram matrix
K K^T. (I-N)^{-1} is formed with Neumann doubling (N is nilpotent), all as
128x128 TensorEngine matmuls. Only the final M is needed downstream (the
reference consumes fused[:, -1, :] only), so per chunk we emit closed-form
state updates M_L, S_L via two more matmuls.
"""

import numpy as np
from contextlib import ExitStack

import sys

for _p in ("/opt/trn_rl_repo",):
    if _p not in sys.path:
        sys.path.append(_p)

B, T, DI, D = 8, 256, 64, 128
PRED_LEN, OUT_DIM = 96, 7
OUTN = PRED_LEN * OUT_DIM  # 672
L = 128
NCHUNK = T // L
LN_EPS = 1e-5

_CACHE = {}


def _build():
    import concourse.bass as bass
    import concourse.tile as tile
    from concourse import bacc, mybir

    f32 = mybir.dt.float32
    AF = mybir.ActivationFunctionType
    OP = mybir.AluOpType

    nc = bacc.Bacc()

    def din(name, shape):
        return nc.declare_dram_parameter(name, shape, f32, isOutput=False)

    xT_d = din("xT", [DI, T])
    Wb_d = din("Wb", [DI, D])
    bb_d = din("bb", [D, 1])
    Wk_d = din("Wk", [D, D])
    Wv_d = din("Wv", [D, D])
    Wq_d = din("Wq", [D, D])
    Wm0_d = din("Wm0", [D, 1])
    Wm1_d = din("Wm1", [D, 1])
    Wm2_d = din("Wm2", [D, 1])
    Wf1_d = din("Wf1", [D, D])
    Wf2_d = din("Wf2", [D, D])
    bf_d = din("bf", [D, 1])
    W1_d = din("W1", [D, D])
    b1_d = din("b1", [D, 1])
    g1_d = din("g1", [1, D])
    be1_d = din("be1", [1, D])
    W2_d = din("W2", [D, OUTN])
    b2_d = din("b2", [1, OUTN])
    mlti_d = din("mlti", [D, D])  # 1.0 where part >= free (lower incl diag)
    muti_d = din("muti", [D, D])  # 1.0 where free >= part (upper incl diag)
    iden_d = din("iden", [D, D])
    out_d = nc.declare_dram_parameter("out", [1, OUTN], f32, isOutput=True)

    with ExitStack() as ctx:
        tc = ctx.enter_context(tile.TileContext(nc))
        cst = ctx.enter_context(tc.tile_pool(name="cst", bufs=1))
        wrk = ctx.enter_context(tc.tile_pool(name="wrk", bufs=2))
        tny = ctx.enter_context(tc.tile_pool(name="tny", bufs=2))
        psA = ctx.enter_context(
            tc.tile_pool(name="psA", bufs=2, space=bass.MemorySpace.PSUM)
        )
        psB = ctx.enter_context(
            tc.tile_pool(name="psB", bufs=4, space=bass.MemorySpace.PSUM)
        )
        psT = ctx.enter_context(
            tc.tile_pool(name="psT", bufs=2, space=bass.MemorySpace.PSUM)
        )

        # ---- load constants to SBUF ----
        def load(dram, shape, tag):
            t = cst.tile(shape, f32, tag=tag)
            nc.gpsimd.dma_start(t[:], dram[:])
            return t

        xT = load(xT_d, [DI, T], "xT")
        Wb = load(Wb_d, [DI, D], "Wb")
        bb = load(bb_d, [D, 1], "bb")
        Wk = load(Wk_d, [D, D], "Wk")
        Wv = load(Wv_d, [D, D], "Wv")
        Wq = load(Wq_d, [D, D], "Wq")
        Wm0 = load(Wm0_d, [D, 1], "Wm0")
        Wm1 = load(Wm1_d, [D, 1], "Wm1")
        Wm2 = load(Wm2_d, [D, 1], "Wm2")
        Wf1 = load(Wf1_d, [D, D], "Wf1")
        Wf2 = load(Wf2_d, [D, D], "Wf2")
        bf = load(bf_d, [D, 1], "bf")
        W1 = load(W1_d, [D, D], "W1")
        b1 = load(b1_d, [D, 1], "b1")
        g1 = load(g1_d, [1, D], "g1")
        be1 = load(be1_d, [1, D], "be1")
        W2 = load(W2_d, [D, OUTN], "W2")
        b2 = load(b2_d, [1, OUTN], "b2")
        mlti = load(mlti_d, [D, D], "mlti")
        muti = load(muti_d, [D, D], "muti")
        iden = load(iden_d, [D, D], "iden")

        ones_col = cst.tile([1, D], f32, tag="ones_col")
        nc.vector.memset(ones_col[:], 1.0)
        one11 = cst.tile([1, 1], f32, tag="one11")
        nc.vector.memset(one11[:], 1.0)
        ones_row = cst.tile([1, T], f32, tag="ones_row")
        nc.vector.memset(ones_row[:], 1.0)

        ncopy = [0]

        def p2s(psum_ap, shape, tag, pool=wrk):
            """PSUM -> SBUF copy, alternating DVE/ACT to balance engines."""
            t = pool.tile(shape, f32, tag=tag)
            if ncopy[0] % 2 == 0:
                nc.vector.tensor_copy(t[:], psum_ap)
            else:
                nc.scalar.copy(t[:], psum_ap)
            ncopy[0] += 1
            return t

        def row_to_col(row_ap, n, tag):
            p = psT.tile([n, 1], f32, tag="tp")
            nc.tensor.matmul(p[:], row_ap, one11[:], start=True, stop=True)
            return p2s(p[:], [n, 1], tag, pool=tny)

        def bcast(row_ap, n, tag):
            """[1,n] row -> [128,n] PSUM broadcast."""
            p = psB.tile([D, n], f32, tag="mm")
            nc.tensor.matmul(p[:], ones_col[:], row_ap, start=True, stop=True)
            return p

        # ---- stage 1: features fT = gelu(Wb^T xT + bb) [D, T] ----
        pf = psA.tile([D, T], f32, tag="big")
        nc.tensor.matmul(pf[:], Wb[:], xT[:], start=True, stop=True)
        fT = cst.tile([D, T], f32, tag="fT")
        nc.scalar.activation(fT[:], pf[:], AF.Gelu_apprx_tanh, bias=bb[:])

        # ---- stage 2: projections ----
        pkT = psA.tile([D, T], f32, tag="big")
        nc.tensor.matmul(pkT[:], Wk[:], fT[:], start=True, stop=True)
        kT = p2s(pkT[:], [D, T], "kT", pool=cst)

        Kc = []
        Vc = []
        for c in range(NCHUNK):
            pk = psB.tile([L, D], f32, tag="mm")
            nc.tensor.matmul(
                pk[:], fT[:, c * L : (c + 1) * L], Wk[:], start=True, stop=True
            )
            Kc.append(p2s(pk[:], [L, D], f"Kc{c}", pool=cst))
            pv = psB.tile([L, D], f32, tag="mm")
            nc.tensor.matmul(
                pv[:], fT[:, c * L : (c + 1) * L], Wv[:], start=True, stop=True
            )
            Vc.append(p2s(pv[:], [L, D], f"Vc{c}", pool=cst))

        # ---- stage 3: meta scalars (rows [1, T]) ----
        pm0 = psT.tile([1, T], f32, tag="tp")
        nc.tensor.matmul(pm0[:], Wm0[:], fT[:], start=True, stop=True)
        th_row = cst.tile([1, T], f32, tag="th_row")
        nc.scalar.activation(th_row[:], pm0[:], AF.Sigmoid)
        nc.scalar.mul(th_row[:], th_row[:], 0.01)

        pm1 = psT.tile([1, T], f32, tag="tp")
        nc.tensor.matmul(pm1[:], Wm1[:], fT[:], start=True, stop=True)
        et_row = tny.tile([1, T], f32, tag="et_row")
        nc.scalar.activation(et_row[:], pm1[:], AF.Sigmoid)
        log_et = tny.tile([1, T], f32, tag="log_et")
        nc.scalar.activation(log_et[:], et_row[:], AF.Ln)

        pm2 = psT.tile([1, T], f32, tag="tp")
        nc.tensor.matmul(pm2[:], Wm2[:], fT[:], start=True, stop=True)
        p_row = tny.tile([1, T], f32, tag="p_row")
        nc.scalar.activation(p_row[:], pm2[:], AF.Sigmoid)
        nc.vector.tensor_scalar(p_row[:], p_row[:], -0.1, 1.0, OP.mult, OP.add)
        log_p = tny.tile([1, T], f32, tag="log_p")
        nc.scalar.activation(log_p[:], p_row[:], AF.Ln)

        # prefix sums (inclusive) with a leading zero -> [1, T+1]
        le_ext = cst.tile([1, T + 1], f32, tag="le_ext")
        nc.vector.memset(le_ext[:, 0:1], 0.0)
        nc.vector.tensor_tensor_scan(
            le_ext[:, 1 : T + 1], ones_row[:], log_et[:], 0.0, OP.mult, OP.add
        )
        la_ext = cst.tile([1, T + 1], f32, tag="la_ext")
        nc.vector.memset(la_ext[:, 0:1], 0.0)
        nc.vector.tensor_tensor_scan(
            la_ext[:, 1 : T + 1], ones_row[:], log_p[:], 0.0, OP.mult, OP.add
        )

        # ---- chunks ----
        MT_sb = None
        ST_sb = None
        for c in range(NCHUNK):
            t0 = c * L
            last = c == NCHUNK - 1
            le_seg = le_ext[:, t0 + 1 : t0 + L + 1]
            la_seg = la_ext[:, t0 + 1 : t0 + L + 1]

            le_col = row_to_col(le_seg, L, f"le_col{c}")
            la_col = row_to_col(la_seg, L, f"la_col{c}")
            th_col = row_to_col(th_row[:, t0 : t0 + L], L, f"th_col{c}")

            # tables: Ftil^T = exp(-max(le_row-le_col,0)) * lower_mask
            #         Gtil   = exp( min(la_row-la_col,0)) * upper_mask
            le_b = bcast(le_seg, L, f"le_b{c}")
            dpos = wrk.tile([L, L], f32, tag="dpos")
            nc.vector.tensor_scalar(
                dpos[:], le_b[:], le_col[:], 0.0, OP.subtract, OP.max
            )
            FtT = wrk.tile([L, L], f32, tag="FtT")
            nc.scalar.activation(FtT[:], dpos[:], AF.Exp, scale=-1.0)
            nc.vector.tensor_mul(FtT[:], FtT[:], mlti[:])

            la_b = bcast(la_seg, L, f"la_b{c}")
            dneg = wrk.tile([L, L], f32, tag="dneg")
            nc.vector.tensor_scalar(
                dneg[:], la_b[:], la_col[:], 0.0, OP.subtract, OP.min
            )
            Gt = wrk.tile([L, L], f32, tag="Gt")
            nc.scalar.activation(Gt[:], dneg[:], AF.Exp)
            nc.vector.tensor_mul(Gt[:], Gt[:], muti[:])

            pC = psB.tile([L, L], f32, tag="mm")
            nc.tensor.matmul(pC[:], FtT[:], Gt[:], start=True, stop=True)

            # Gram matrix Psi = K K^T
            pPsi = psB.tile([L, L], f32, tag="mm")
            nc.tensor.matmul(
                pPsi[:],
                kT[:, t0 : t0 + L],
                kT[:, t0 : t0 + L],
                start=True,
                stop=True,
            )

            # C shifted right in free dim; col 0 zero. C[j,tau]=0 for tau<j
            # already, so C_sh is strictly-upper by construction.
            C_sh = wrk.tile([L, L], f32, tag="C_sh")
            nc.vector.memset(C_sh[:, 0:1], 0.0)
            nc.vector.tensor_copy(C_sh[:, 1:L], pC[:, 0 : L - 1])
            cL_col = p2s(pC[:, L - 1 : L], [L, 1], f"cL{c}", pool=tny)

            # NT[j,t] = -th_t * C[j,t-1] * Psi[j,t]  (strictly upper)
            NT_a = wrk.tile([L, L], f32, tag="NT_a")
            nc.vector.tensor_mul(NT_a[:], C_sh[:], pPsi[:])
            th_b = bcast(th_row[:, t0 : t0 + L], L, f"th_b{c}")
            NT = wrk.tile([L, L], f32, tag="NT")
            nc.vector.scalar_tensor_tensor(
                NT[:], th_b[:], -1.0, NT_a[:], OP.mult, OP.mult
            )

            # N = NT^T via PE transpose
            pN = psB.tile([L, L], f32, tag="mm")
            nc.tensor.transpose(pN[:], NT[:], iden[:])
            X = p2s(pN[:], [L, L], "Xk", pool=wrk)
            Y = NT

            # INVT = (I - NT)^{-1} built by Neumann doubling (transposed so
            # W = INV @ R becomes matmul(lhsT=INVT, rhs=R)).
            INVT = wrk.tile([L, L], f32, tag="INVT")
            nc.vector.tensor_add(INVT[:], NT[:], iden[:])
            for lev in range(1, 7):
                pX2 = psA.tile([L, L], f32, tag="big")
                nc.tensor.matmul(pX2[:], Y[:], X[:], start=True, stop=True)
                X2 = p2s(pX2[:], [L, L], "Xk", pool=wrk)
                if lev < 6:
                    pY2 = psA.tile([L, L], f32, tag="big")
                    nc.tensor.matmul(pY2[:], X[:], Y[:], start=True, stop=True)
                    Y = p2s(pY2[:], [L, L], "Yk", pool=wrk)
                X = X2
                pIU = psA.tile([L, L], f32, tag="big")
                nc.tensor.matmul(pIU[:], X[:], INVT[:], start=True, stop=True)
                INVT2 = wrk.tile([L, L], f32, tag="INVT")
                nc.vector.tensor_add(INVT2[:], INVT[:], pIU[:])
                INVT = INVT2

            # R
            if c == 0:
                R = wrk.tile([L, D], f32, tag="R")
                nc.vector.tensor_scalar(R[:], Vc[c][:], th_col[:], None, OP.mult)
            else:
                la_prev_col = row_to_col(
                    la_ext[:, t0 : t0 + L], L, f"la_prev{c}"
                )
                # neg la0 / le0 broadcast columns for exp biases
                nla0 = psT.tile([D, 1], f32, tag="tp")
                nc.tensor.matmul(
                    nla0[:], ones_col[:], la_ext[:, t0 : t0 + 1], start=True, stop=True
                )
                nla0_sb = tny.tile([D, 1], f32, tag="nla0_sb")
                nc.scalar.mul(nla0_sb[:], nla0[:], -1.0)
                A_prev = tny.tile([L, 1], f32, tag="A_prev")
                nc.scalar.activation(
                    A_prev[:], la_prev_col[:], AF.Exp, bias=nla0_sb[:]
                )

                nle0 = psT.tile([D, 1], f32, tag="tp")
                nc.tensor.matmul(
                    nle0[:], ones_col[:], le_ext[:, t0 : t0 + 1], start=True, stop=True
                )
                nle0_sb = tny.tile([D, 1], f32, tag="nle0_sb")
                nc.scalar.mul(nle0_sb[:], nle0[:], -1.0)
                E_col = tny.tile([L, 1], f32, tag="E_col")
                nc.scalar.activation(E_col[:], le_col[:], AF.Exp, bias=nle0_sb[:])

                # b row = E_col^T @ Gtil ; b_prev = shifted
                pb = psT.tile([1, L], f32, tag="tp")
                nc.tensor.matmul(pb[:], E_col[:], Gt[:], start=True, stop=True)
                b_row = p2s(pb[:], [1, L], "b_row", pool=tny)
                b_sh = tny.tile([1, L], f32, tag="b_sh")
                nc.vector.memset(b_sh[:, 0:1], 0.0)
                nc.vector.tensor_copy(b_sh[:, 1:L], b_row[:, 0 : L - 1])
                b_prev = row_to_col(b_sh[:], L, f"b_prev{c}")

                # A_L, b_L broadcast columns (scalars of this chunk)
                dl = tny.tile([1, 1], f32, tag="dl")
                nc.vector.tensor_scalar(
                    dl[:],
                    la_ext[:, t0 + L : t0 + L + 1],
                    la_ext[:, t0 : t0 + 1],
                    None,
                    OP.subtract,
                )
                nc.scalar.activation(dl[:], dl[:], AF.Exp)
                pAL = psT.tile([D, 1], f32, tag="tp")
                nc.tensor.matmul(pAL[:], ones_col[:], dl[:], start=True, stop=True)
                AL_col = p2s(pAL[:], [D, 1], "AL_col", pool=tny)
                pbL = psT.tile([D, 1], f32, tag="tp")
                nc.tensor.matmul(
                    pbL[:], ones_col[:], b_row[:, L - 1 : L], start=True, stop=True
                )
                bL_col = p2s(pbL[:], [D, 1], "bL_col", pool=tny)

                pZM = psA.tile([L, D], f32, tag="big")
                nc.tensor.matmul(
                    pZM[:], kT[:, t0 : t0 + L], MT_sb[:], start=True, stop=True
                )
                pZS = psA.tile([L, D], f32, tag="big")
                nc.tensor.matmul(
                    pZS[:], kT[:, t0 : t0 + L], ST_sb[:], start=True, stop=True
                )
                t1 = wrk.tile([L, D], f32, tag="t1")
                nc.vector.tensor_scalar(t1[:], pZM[:], A_prev[:], None, OP.mult)
                t2 = wrk.tile([L, D], f32, tag="t2")
                nc.vector.scalar_tensor_tensor(
                    t2[:], pZS[:], b_prev[:], t1[:], OP.mult, OP.add
                )
                nc.vector.tensor_sub(t2[:], t2[:], Vc[c][:])
                R = wrk.tile([L, D], f32, tag="R")
                nc.vector.tensor_scalar(
                    R[:], t2[:], th_col[:], -1.0, OP.mult, OP.mult
                )

            # W = INV @ R
            pW = psA.tile([L, D], f32, tag="big")
            nc.tensor.matmul(pW[:], INVT[:], R[:], start=True, stop=True)
            W = p2s(pW[:], [L, D], "W", pool=wrk)

            # state update
            Wp = wrk.tile([L, D], f32, tag="Wp")
            nc.vector.tensor_scalar(Wp[:], W[:], cL_col[:], None, OP.mult)
            pMTc = psA.tile([D, D], f32, tag="big")
            nc.tensor.matmul(pMTc[:], Kc[c][:], Wp[:], start=True, stop=True)

            if c == 0:
                MT_sb = p2s(pMTc[:], [D, D], "MT", pool=cst)
                # S update needed only when a later chunk consumes it
                leL_b = psT.tile([D, 1], f32, tag="tp")
                nc.tensor.matmul(
                    leL_b[:],
                    ones_col[:],
                    le_ext[:, t0 + L : t0 + L + 1],
                    start=True,
                    stop=True,
                )
                leL_sb = p2s(leL_b[:], [D, 1], "leL_sb", pool=tny)
                FL_col = tny.tile([L, 1], f32, tag="FL_col")
                nc.scalar.activation(
                    FL_col[:], le_col[:], AF.Exp, scale=-1.0, bias=leL_sb[:]
                )
                Wpp = wrk.tile([L, D], f32, tag="Wpp")
                nc.vector.tensor_scalar(Wpp[:], W[:], FL_col[:], None, OP.mult)
                pSTc = psA.tile([D, D], f32, tag="big")
                nc.tensor.matmul(pSTc[:], Kc[c][:], Wpp[:], start=True, stop=True)
                ST_sb = p2s(pSTc[:], [D, D], "ST", pool=cst)
            else:
                a1 = wrk.tile([D, D], f32, tag="a1")
                nc.vector.scalar_tensor_tensor(
                    a1[:], MT_sb[:], AL_col[:], pMTc[:], OP.mult, OP.add
                )
                MT2 = wrk.tile([D, D], f32, tag="MT2")
                nc.vector.scalar_tensor_tensor(
                    MT2[:], ST_sb[:], bL_col[:], a1[:], OP.mult, OP.add
                )
                MT_sb = MT2

        # ---- head (last timestep only) ----
        f_last = fT[:, T - 1 : T]
        pq = psT.tile([D, 1], f32, tag="tp")
        nc.tensor.matmul(pq[:], Wq[:], f_last, start=True, stop=True)
        q_col = p2s(pq[:], [D, 1], "q_col", pool=tny)

        pmm = psT.tile([D, 1], f32, tag="tp")
        nc.tensor.matmul(pmm[:], MT_sb[:], q_col[:], start=True, stop=True)
        m_col = p2s(pmm[:], [D, 1], "m_col", pool=tny)

        pg = psT.tile([D, 1], f32, tag="tp")
        nc.tensor.matmul(pg[:], Wf1[:], f_last, start=True, stop=False)
        nc.tensor.matmul(pg[:], Wf2[:], m_col[:], start=False, stop=True)
        gate = tny.tile([D, 1], f32, tag="gate")
        nc.scalar.activation(gate[:], pg[:], AF.Sigmoid, bias=bf[:])

        dfm = tny.tile([D, 1], f32, tag="dfm")
        nc.vector.tensor_sub(dfm[:], f_last, m_col[:])
        fused = tny.tile([D, 1], f32, tag="fused")
        nc.vector.scalar_tensor_tensor(
            fused[:], dfm[:], gate[:], m_col[:], OP.mult, OP.add
        )

        py = psT.tile([D, 1], f32, tag="tp")
        nc.tensor.matmul(py[:], W1[:], fused[:], start=True, stop=True)
        y_col = tny.tile([D, 1], f32, tag="y_col")
        nc.scalar.activation(y_col[:], py[:], AF.Identity, bias=b1[:])

        pyr = psT.tile([1, D], f32, tag="tp")
        nc.tensor.matmul(pyr[:], y_col[:], iden[:], start=True, stop=True)
        y_row = tny.tile([1, D], f32, tag="y_row")
        nc.vector.tensor_copy(y_row[:], pyr[:])

        mu = tny.tile([1, 1], f32, tag="mu")
        nc.vector.tensor_reduce(mu[:], y_row[:], mybir.AxisListType.X, OP.add)
        nc.scalar.mul(mu[:], mu[:], 1.0 / D)
        xc = tny.tile([1, D], f32, tag="xc")
        nc.vector.tensor_scalar(xc[:], y_row[:], mu[:], None, OP.subtract)
        sq = tny.tile([1, D], f32, tag="sq")
        nc.vector.tensor_mul(sq[:], xc[:], xc[:])
        var = tny.tile([1, 1], f32, tag="var")
        nc.vector.tensor_reduce(var[:], sq[:], mybir.AxisListType.X, OP.add)
        eps_t = tny.tile([1, 1], f32, tag="eps_t")
        nc.vector.memset(eps_t[:], LN_EPS)
        sd = tny.tile([1, 1], f32, tag="sd")
        nc.scalar.activation(sd[:], var[:], AF.Sqrt, scale=1.0 / D, bias=eps_t[:])
        rstd = tny.tile([1, 1], f32, tag="rstd")
        nc.vector.reciprocal(rstd[:], sd[:])

        hh = tny.tile([1, D], f32, tag="hh")
        nc.vector.tensor_scalar(hh[:], xc[:], rstd[:], None, OP.mult)
        nc.vector.tensor_mul(hh[:], hh[:], g1[:])
        nc.vector.tensor_add(hh[:], hh[:], be1[:])
        h_row = tny.tile([1, D], f32, tag="h_row")
        nc.scalar.activation(h_row[:], hh[:], AF.Gelu_apprx_tanh)

        ph = psT.tile([D, 1], f32, tag="tp")
        nc.tensor.matmul(ph[:], h_row[:], one11[:], start=True, stop=True)
        h_col = p2s(ph[:], [D, 1], "h_col", pool=tny)

        po1 = psT.tile([1, 512], f32, tag="tp")
        nc.tensor.matmul(po1[:], h_col[:], W2[:, 0:512], start=True, stop=True)
        po2 = psT.tile([1, OUTN - 512], f32, tag="tp")
        nc.tensor.matmul(po2[:], h_col[:], W2[:, 512:OUTN], start=True, stop=True)
        orow = tny.tile([1, OUTN], f32, tag="orow")
        nc.vector.tensor_add(orow[:, 0:512], po1[:], b2[:, 0:512])
        nc.vector.tensor_add(orow[:, 512:OUTN], po2[:], b2[:, 512:OUTN])

        nc.gpsimd.dma_start(out_d[:], orow[:])

    nc.finalize()
    return nc


def _prep_maps(inputs):
    f = np.float32
    x = np.asarray(inputs["x"], f)
    idx = np.arange(D)
    mlti = (idx[:, None] >= idx[None, :]).astype(f)
    muti = (idx[None, :] >= idx[:, None]).astype(f)
    iden = np.eye(D, dtype=f)
    base = {
        "Wb": np.ascontiguousarray(np.asarray(inputs["W_b"], f)),
        "bb": np.asarray(inputs["b_b"], f).reshape(D, 1).copy(),
        "Wk": np.ascontiguousarray(np.asarray(inputs["Wk"], f)),
        "Wv": np.ascontiguousarray(np.asarray(inputs["Wv"], f)),
        "Wq": np.ascontiguousarray(np.asarray(inputs["Wq"], f)),
        "Wm0": np.asarray(inputs["W_m"], f)[:, 0:1].copy(),
        "Wm1": np.asarray(inputs["W_m"], f)[:, 1:2].copy(),
        "Wm2": np.asarray(inputs["W_m"], f)[:, 2:3].copy(),
        "Wf1": np.ascontiguousarray(np.asarray(inputs["W_f"], f)[:D]),
        "Wf2": np.ascontiguousarray(np.asarray(inputs["W_f"], f)[D:]),
        "bf": np.asarray(inputs["b_f"], f).reshape(D, 1).copy(),
        "W1": np.ascontiguousarray(np.asarray(inputs["W1"], f)),
        "b1": np.asarray(inputs["b1"], f).reshape(D, 1).copy(),
        "g1": np.asarray(inputs["g1"], f).reshape(1, D).copy(),
        "be1": np.asarray(inputs["be1"], f).reshape(1, D).copy(),
        "W2": np.ascontiguousarray(np.asarray(inputs["W2"], f)),
        "b2": np.asarray(inputs["b2"], f).reshape(1, OUTN).copy(),
        "mlti": mlti,
        "muti": muti,
        "iden": iden,
    }
    maps = []
    for b in range(B):
        m = dict(base)
        m["xT"] = np.ascontiguousarray(x[b].T)
        maps.append(m)
    return maps


def kernel(**inputs):
    from concourse.bass_utils import run_bass_kernel_spmd

    if "nc" not in _CACHE:
        _CACHE["nc"] = _build()
    nc = _CACHE["nc"]
    maps = _prep_maps(inputs)
    res = run_bass_kernel_spmd(nc, maps, core_ids=list(range(B)))
    outs = [res.results[i]["out"].reshape(PRED_LEN, OUT_DIM) for i in range(B)]
    return np.stack(outs).astype(np.float32)



# revision 5
# speedup vs baseline: 1.4945x; 1.4945x over previous
"""Trainium2 Bass kernel for nn_ContinualForecaster (scatter_memory).

Data-parallel over batch (B=8 -> 8 NeuronCores). The T=256 recurrence on
M,S [128,128] is a chunked parallel scan (2 chunks of L=128) solving the
strictly-triangular system W = (I-N)^{-1} R via Neumann doubling.

v2 optimizations vs baseline:
- fp16 matmul operands everywhere except the meta/backbone path (PE runs
  fp16 at 1 cycle/row vs fp32's 4; halves LDWEIGHTS passes too).
- Neumann doubling truncated to 4 levels (N^16 is ~0.3^16, numerically
  validated: rel err 9.6e-7 vs 6-level baseline).
- C computed pre-transposed+pre-shifted via matmul(Gt_shifted, FtT), so N
  is assembled directly with a per-partition th column (no th broadcast,
  no separate NT assembly).
- Fused matmuls sharing a stationary operand: meta [D,3], [Wk|Wv] [D,2D],
  [M|S] ZMS [D,2D].
- Only 4 ACT table loads (Gelu/Sigmoid/Ln/Exp), each preloaded by a dummy
  op so the 1.28us load hides behind other engines; the whole tail
  (gate sigmoid, gelu, rsqrt) uses the resident Exp table + DVE
  reciprocal + gpsimd pow(-0.5).
- Weights packed into 3 DMA blobs issued on separate queues.
"""

import numpy as np
from contextlib import ExitStack

import sys

for _p in ("/opt/trn_rl_repo",):
    if _p not in sys.path:
        sys.path.append(_p)

B, T, DI, D = 8, 256, 64, 128
PRED_LEN, OUT_DIM = 96, 7
OUTN = PRED_LEN * OUT_DIM  # 672
L = 128
NCHUNK = T // L
NLEV = 4
LN_EPS = 1e-5
GC2 = 2.0 * 0.7978845608028654  # 2*sqrt(2/pi)

# blob16 column offsets (fp16 weights, [D, C16])
C_WKV = 0      # [Wk | Wv]  256
C_WQ = 256     # Wq         128
C_W1 = 384     # W1         128
C_WFA = 512    # W_f[:D]    128
C_WFB = 640    # W_f[D:]    128
C_W2 = 768     # W2         672
C_MLT = 1440   # lower-incl mask 128
C_MUT = 1568   # upper-incl mask 128
C_IDE = 1696   # identity   128
C16 = 1824

# blob32 column offsets (fp32, [128, C32]); Wb occupies partitions 0:64
B_WB = 0       # Wb   128
B_WM = 128     # W_m spread to cols 0/32/64 of a 65-wide block
B_BB = 193     # b_b  1
B_NBF = 194    # -b_f 1
C32 = 195

# rows32 offsets ([1, NR] fp32)
R_B1 = 0       # b1   128
R_G1 = 128     # g1   128
R_BE1 = 256    # be1  128
R_B2 = 384     # b2   672
NR = 1056

_CACHE = {}


def _build():
    import concourse.bass as bass
    import concourse.tile as tile
    from concourse import bacc, mybir

    f32 = mybir.dt.float32
    f16 = mybir.dt.float16
    AF = mybir.ActivationFunctionType
    OP = mybir.AluOpType

    nc = bacc.Bacc()

    xT_d = nc.declare_dram_parameter("xT", [DI, T], f32, isOutput=False)
    b32_d = nc.declare_dram_parameter("b32", [D, C32], f32, isOutput=False)
    b16_d = nc.declare_dram_parameter("b16", [D, C16], f16, isOutput=False)
    rws_d = nc.declare_dram_parameter("rws", [1, NR], f32, isOutput=False)
    out_d = nc.declare_dram_parameter("out", [1, OUTN], f32, isOutput=True)

    with ExitStack() as ctx:
        tc = ctx.enter_context(tile.TileContext(nc))
        cst = ctx.enter_context(tc.tile_pool(name="cst", bufs=1))
        wrk = ctx.enter_context(tc.tile_pool(name="wrk", bufs=2))
        tny = ctx.enter_context(tc.tile_pool(name="tny", bufs=2))
        psA = ctx.enter_context(
            tc.tile_pool(name="psA", bufs=2, space=bass.MemorySpace.PSUM)
        )
        psB = ctx.enter_context(
            tc.tile_pool(name="psB", bufs=4, space=bass.MemorySpace.PSUM)
        )
        psT = ctx.enter_context(
            tc.tile_pool(name="psT", bufs=2, space=bass.MemorySpace.PSUM)
        )

        # ---- tiny constants (DVE memsets run at t=0) ----
        dm = cst.tile([1, 1], f32, tag="dm")
        nc.vector.memset(dm[:], 0.5)
        du = cst.tile([1, 1], f32, tag="du")
        ones1 = cst.tile([1, D], f32, tag="ones1")
        nc.vector.memset(ones1[:], 1.0)
        one11 = cst.tile([1, 1], f32, tag="one11")
        nc.vector.memset(one11[:], 1.0)
        one11h = cst.tile([1, 1], f16, tag="one11h")
        nc.vector.memset(one11h[:], 1.0)
        ones_row = cst.tile([1, T], f32, tag="ones_row")
        nc.vector.memset(ones_row[:], 1.0)
        mhalf = cst.tile([1, 1], f32, tag="mhalf")
        nc.vector.memset(mhalf[:], -0.5)

        # ---- input DMAs on separate queues ----
        xT = cst.tile([DI, T], f32, tag="xT")
        b32 = cst.tile([D, C32], f32, tag="b32")
        b16 = cst.tile([D, C16], f16, tag="b16")
        rws = cst.tile([1, NR], f32, tag="rws")
        nc.sync.dma_start(xT[:], xT_d[:])
        nc.sync.dma_start(b32[:], b32_d[:])
        nc.gpsimd.dma_start(b16[:], b16_d[:])
        nc.gpsimd.dma_start(rws[:], rws_d[:])

        # dummy Gelu preloads the table during the DMAs
        nc.scalar.activation(du[:], dm[:], AF.Gelu_apprx_tanh)

        mlti = b16[:, C_MLT : C_MLT + D]
        muti = b16[:, C_MUT : C_MUT + D]
        iden = b16[:, C_IDE : C_IDE + D]

        ncopy = [0]

        def p2s(psum_ap, shape, tag, pool=wrk, dt=f16):
            """PSUM -> SBUF copy (casting), alternating DVE/ACT."""
            t = pool.tile(shape, dt, tag=tag)
            if ncopy[0] % 2 == 0:
                nc.vector.tensor_copy(t[:], psum_ap)
            else:
                nc.scalar.copy(t[:], psum_ap)
            ncopy[0] += 1
            return t

        def row_to_col(row_ap, n, tag, dt=f32):
            p = psT.tile([n, 1], f32, tag="t")
            nc.tensor.matmul(p[:], row_ap, one11[:], start=True, stop=True)
            return p2s(p[:], [n, 1], tag, pool=tny, dt=dt)

        # ---- stage 1: fT = gelu(Wb^T xT + bb)  [D, T] fp32 ----
        pf = psA.tile([D, T], f32, tag="A")
        nc.tensor.matmul(pf[:], b32[0:DI, B_WB : B_WB + D], xT[:], start=True, stop=True)
        fT = cst.tile([D, T], f32, tag="fT")
        nc.scalar.activation(fT[:], pf[:], AF.Gelu_apprx_tanh, bias=b32[:, B_BB : B_BB + 1])
        nc.scalar.activation(du[:], dm[:], AF.Sigmoid)  # preload
        fT16 = cst.tile([D, T], f16, tag="fT16")
        nc.vector.tensor_copy(fT16[:], fT[:])

        # ---- meta: [3, T] = Wm^T fT; sigmoid ----
        pmt = psT.tile([65, T], f32, tag="t")
        nc.tensor.matmul(pmt[:], b32[:, B_WM : B_WM + 65], fT[:], start=True, stop=True)
        msg = cst.tile([65, T], f32, tag="msg")
        nc.scalar.activation(msg[:], pmt[:], AF.Sigmoid)
        nc.scalar.activation(du[:], dm[:], AF.Ln)  # preload
        th_row = cst.tile([1, T], f32, tag="th_row")
        nc.vector.tensor_scalar(th_row[:], msg[0:1, :], 0.01, None, OP.mult)
        p_row = cst.tile([1, T], f32, tag="p_row")
        nc.vector.tensor_scalar(p_row[:], msg[64:65, :], -0.1, 1.0, OP.mult, OP.add)
        log_et = cst.tile([1, T], f32, tag="log_et")
        nc.scalar.activation(log_et[:], msg[32:33, :], AF.Ln)
        log_p = cst.tile([1, T], f32, tag="log_p")
        nc.scalar.activation(log_p[:], p_row[:], AF.Ln)
        nc.scalar.activation(du[:], dm[:], AF.Exp)  # preload; last table switch

        # prefix sums with leading zero -> [1, T+1]
        le_ext = cst.tile([1, T + 1], f32, tag="le_ext")
        nc.vector.memset(le_ext[:, 0:1], 0.0)
        nc.vector.tensor_tensor_scan(
            le_ext[:, 1 : T + 1], ones_row[:], log_et[:], 0.0, OP.mult, OP.add
        )
        la_ext = cst.tile([1, T + 1], f32, tag="la_ext")
        nc.vector.memset(la_ext[:, 0:1], 0.0)
        nc.vector.tensor_tensor_scan(
            la_ext[:, 1 : T + 1], ones_row[:], log_p[:], 0.0, OP.mult, OP.add
        )

        # ---- projections (fp16) ----
        pk = psA.tile([D, T], f32, tag="A")
        nc.tensor.matmul(pk[:], b16[:, C_WKV : C_WKV + D], fT16[:], start=True, stop=True)
        kT = p2s(pk[:], [D, T], "kT", pool=cst)
        KV = []
        for c in range(NCHUNK):
            pkv = psA.tile([L, 2 * D], f32, tag="A")
            nc.tensor.matmul(
                pkv[:], fT16[:, c * L : (c + 1) * L], b16[:, C_WKV : C_WKV + 2 * D],
                start=True, stop=True,
            )
            KV.append(p2s(pkv[:], [L, 2 * D], f"KV{c}", pool=cst))
        pq = psT.tile([D, 1], f32, tag="t")
        nc.tensor.matmul(pq[:], b16[:, C_WQ : C_WQ + D], fT16[:, T - 1 : T], start=True, stop=True)
        q16 = p2s(pq[:], [D, 1], "q16", pool=tny)

        # ---- per-chunk scalar columns (PE transposes) ----
        le_col, la_col, th_col, negth = [], [], [], []
        for c in range(NCHUNK):
            t0 = c * L
            le_col.append(row_to_col(le_ext[:, t0 + 1 : t0 + L + 1], L, f"lec{c}"))
            la_col.append(row_to_col(la_ext[:, t0 + 1 : t0 + L + 1], L, f"lac{c}"))
            th_col.append(row_to_col(th_row[:, t0 : t0 + L], L, f"thc{c}"))
            nt = tny.tile([L, 1], f32, tag=f"nth{c}")
            nc.vector.tensor_scalar(nt[:], th_col[c][:], -1.0, None, OP.mult)
            negth.append(nt)

        # FL0 col for chunk0 S update: exp(le_L - le_t)
        dfl = tny.tile([1, L], f32, tag="dfl")
        nc.vector.tensor_scalar(dfl[:], le_ext[:, 1 : L + 1], le_ext[:, L : L + 1], None, OP.subtract)
        pfl = psT.tile([L, 1], f32, tag="t")
        nc.tensor.matmul(pfl[:], dfl[:], one11[:], start=True, stop=True)
        FL_col = tny.tile([L, 1], f32, tag="FL_col")
        nc.scalar.activation(FL_col[:], pfl[:], AF.Exp, scale=-1.0)

        # chunk1 R prep: A_prev = exp(la_{t-1}-la_0), E_col = exp(le_t-le_0)
        t1_0 = L
        dla = tny.tile([1, L], f32, tag="dla")
        nc.vector.tensor_scalar(dla[:], la_ext[:, t1_0 : t1_0 + L], la_ext[:, t1_0 : t1_0 + 1], None, OP.subtract)
        pap = psT.tile([L, 1], f32, tag="t")
        nc.tensor.matmul(pap[:], dla[:], one11[:], start=True, stop=True)
        A_prev = tny.tile([L, 1], f32, tag="A_prev")
        nc.scalar.activation(A_prev[:], pap[:], AF.Exp)
        dle = tny.tile([1, L], f32, tag="dle")
        nc.vector.tensor_scalar(dle[:], le_ext[:, t1_0 + 1 : t1_0 + L + 1], le_ext[:, t1_0 : t1_0 + 1], None, OP.subtract)
        pec = psT.tile([L, 1], f32, tag="t")
        nc.tensor.matmul(pec[:], dle[:], one11[:], start=True, stop=True)
        E_col = tny.tile([L, 1], f16, tag="E_col")
        nc.scalar.activation(E_col[:], pec[:], AF.Exp)
        # AL = exp(la_L1 - la_0) broadcast to [D,1]
        dls = tny.tile([1, 1], f32, tag="dls")
        nc.vector.tensor_scalar(dls[:], la_ext[:, t1_0 + L : t1_0 + L + 1], la_ext[:, t1_0 : t1_0 + 1], None, OP.subtract)
        als = tny.tile([1, 1], f32, tag="als")
        nc.scalar.activation(als[:], dls[:], AF.Exp)
        pal = psT.tile([D, 1], f32, tag="t")
        nc.tensor.matmul(pal[:], ones1[:], als[:], start=True, stop=True)
        AL_col = p2s(pal[:], [D, 1], "AL_col", pool=tny, dt=f32)

        # ---- tables + N per chunk ----
        Gtm = [None, None]
        N32 = [None, None]
        cL_col = [None, None]
        for c in range(NCHUNK):
            t0 = c * L
            le_seg = le_ext[:, t0 + 1 : t0 + L + 1]
            la_seg = la_ext[:, t0 + 1 : t0 + L + 1]

            pleb = psB.tile([D, L], f32, tag="B")
            nc.tensor.matmul(pleb[:], ones1[:], le_seg, start=True, stop=True)
            dpos = wrk.tile([L, L], f32, tag=f"dpos{c}")
            nc.vector.tensor_scalar(dpos[:], pleb[:], le_col[c][:], 0.0, OP.subtract, OP.max)
            FtT = wrk.tile([L, L], f16, tag=f"FtT{c}")
            nc.scalar.activation(FtT[:], dpos[:], AF.Exp, scale=-1.0)
            FtTm = wrk.tile([L, L], f16, tag=f"FtTm{c}")
            nc.vector.tensor_mul(FtTm[:], FtT[:], mlti)

            plab = psB.tile([D, L], f32, tag="B")
            nc.tensor.matmul(plab[:], ones1[:], la_seg, start=True, stop=True)
            dneg = wrk.tile([L, L], f32, tag=f"dneg{c}")
            nc.vector.tensor_scalar(dneg[:], plab[:], la_col[c][:], 0.0, OP.subtract, OP.min)
            Gt = wrk.tile([L, L], f16, tag=f"Gt{c}")
            nc.scalar.activation(Gt[:], dneg[:], AF.Exp)
            gtm = wrk.tile([L, L], f16, tag=f"Gtm{c}")
            nc.vector.tensor_mul(gtm[:], Gt[:], muti)
            Gtm[c] = gtm

            gsh = wrk.tile([L, L], f16, tag=f"Gsh{c}")
            nc.vector.memset(gsh[:, 0:1], 0.0)
            nc.vector.tensor_copy(gsh[:, 1:L], gtm[:, 0 : L - 1])

            pct = psB.tile([L, L], f32, tag="B")
            nc.tensor.matmul(pct[:], gsh[:], FtTm[:], start=True, stop=True)
            ppsi = psB.tile([L, L], f32, tag="B")
            nc.tensor.matmul(
                ppsi[:], kT[:, t0 : t0 + L], kT[:, t0 : t0 + L], start=True, stop=True
            )
            psi16 = wrk.tile([L, L], f16, tag=f"psi{c}")
            nc.scalar.copy(psi16[:], ppsi[:])

            pcl = psT.tile([L, 1], f32, tag="t")
            nc.tensor.matmul(pcl[:], FtTm[:], gtm[:, L - 1 : L], start=True, stop=True)
            cL_col[c] = p2s(pcl[:], [L, 1], f"cL{c}", pool=tny, dt=f32)

            n32 = wrk.tile([L, L], f32, tag=f"N32{c}")
            nc.vector.scalar_tensor_tensor(
                n32[:], pct[:], negth[c][:], psi16[:], OP.mult, OP.mult
            )
            N32[c] = n32

        # R0 (ready for the post-doubling solve)
        R0 = wrk.tile([L, D], f16, tag="R0")
        nc.vector.tensor_scalar(R0[:], KV[0][:, D : 2 * D], th_col[0][:], None, OP.mult)

        # ---- transpose + Neumann doubling per chunk ----
        MS16 = cst.tile([D, 2 * D], f16, tag="MS16")

        def doubling_fixed(c):
            pnt = psB.tile([L, L], f32, tag="B")
            # fp32 transpose: identity must be fp32 to match in_ dtype
            nc.tensor.transpose(pnt[:], N32[c][:], iden32[:])
            X = wrk.tile([L, L], f16, tag=f"X{c}")
            nc.vector.tensor_copy(X[:], N32[c][:])
            Y = wrk.tile([L, L], f16, tag=f"Y{c}")
            nc.scalar.copy(Y[:], pnt[:])
            IV = wrk.tile([L, L], f16, tag=f"IV{c}")
            nc.vector.tensor_add(IV[:], pnt[:], iden)
            for lev in range(1, NLEV + 1):
                px2 = psB.tile([L, L], f32, tag="B")
                nc.tensor.matmul(px2[:], Y[:], X[:], start=True, stop=True)
                X2 = p2s(px2[:], [L, L], f"X{c}")
                if lev < NLEV:
                    py2 = psB.tile([L, L], f32, tag="B")
                    nc.tensor.matmul(py2[:], X[:], Y[:], start=True, stop=True)
                    Y = p2s(py2[:], [L, L], f"Y{c}")
                X = X2
                piu = psB.tile([L, L], f32, tag="B")
                nc.tensor.matmul(piu[:], X[:], IV[:], start=True, stop=True)
                IV2 = wrk.tile([L, L], f16, tag=f"IV{c}")
                nc.vector.tensor_add(IV2[:], IV[:], piu[:])
                IV = IV2
            return IV

        iden32 = cst.tile([D, D], f32, tag="iden32")
        nc.vector.tensor_copy(iden32[:], iden)

        IV0 = doubling_fixed(0)

        # chunk0 solve + state
        pw0 = psA.tile([L, D], f32, tag="A")
        nc.tensor.matmul(pw0[:], IV0[:], R0[:], start=True, stop=True)
        W0 = p2s(pw0[:], [L, D], "W0")
        Wp0 = wrk.tile([L, D], f16, tag="Wp0")
        nc.vector.tensor_scalar(Wp0[:], W0[:], cL_col[0][:], None, OP.mult)
        pmt0 = psA.tile([D, D], f32, tag="A")
        nc.tensor.matmul(pmt0[:], KV[0][:, 0:D], Wp0[:], start=True, stop=True)
        nc.vector.tensor_copy(MS16[:, 0:D], pmt0[:])
        Wpp0 = wrk.tile([L, D], f16, tag="Wpp0")
        nc.vector.tensor_scalar(Wpp0[:], W0[:], FL_col[:], None, OP.mult)
        pst0 = psA.tile([D, D], f32, tag="A")
        nc.tensor.matmul(pst0[:], KV[0][:, 0:D], Wpp0[:], start=True, stop=True)
        nc.scalar.copy(MS16[:, D : 2 * D], pst0[:])

        # chunk1 b-row pieces (need Gtm[1] + E_col)
        pb = psT.tile([1, L], f32, tag="t")
        nc.tensor.matmul(pb[:], E_col[:], Gtm[1][:], start=True, stop=True)
        b_sh = tny.tile([1, L], f32, tag="b_sh")
        nc.vector.memset(b_sh[:, 0:1], 0.0)
        nc.vector.tensor_copy(b_sh[:, 1:L], pb[:, 0 : L - 1])
        bls = tny.tile([1, 1], f32, tag="bls")
        nc.vector.tensor_copy(bls[:], pb[:, L - 1 : L])
        pbp = psT.tile([L, 1], f32, tag="t")
        nc.tensor.matmul(pbp[:], b_sh[:], one11[:], start=True, stop=True)
        b_prev = p2s(pbp[:], [L, 1], "b_prev", pool=tny, dt=f32)
        pbl = psT.tile([D, 1], f32, tag="t")
        nc.tensor.matmul(pbl[:], ones1[:], bls[:], start=True, stop=True)
        bL_col = p2s(pbl[:], [D, 1], "bL_col", pool=tny, dt=f32)

        # ZMS = K1 [M^T | S^T]
        pzms = psA.tile([L, 2 * D], f32, tag="A")
        nc.tensor.matmul(pzms[:], kT[:, t1_0 : t1_0 + L], MS16[:], start=True, stop=True)
        u1 = wrk.tile([L, D], f32, tag="u1")
        nc.vector.scalar_tensor_tensor(
            u1[:], pzms[:, 0:D], A_prev[:], KV[1][:, D : 2 * D], OP.mult, OP.subtract
        )
        w1t = wrk.tile([L, D], f32, tag="w1t")
        nc.vector.scalar_tensor_tensor(
            w1t[:], pzms[:, D : 2 * D], b_prev[:], u1[:], OP.mult, OP.add
        )
        R1 = wrk.tile([L, D], f16, tag="R1")
        nc.vector.tensor_scalar(R1[:], w1t[:], th_col[1][:], -1.0, OP.mult, OP.mult)

        IV1 = doubling_fixed(1)

        # chunk1 solve + final M
        pw1 = psA.tile([L, D], f32, tag="A")
        nc.tensor.matmul(pw1[:], IV1[:], R1[:], start=True, stop=True)
        W1 = p2s(pw1[:], [L, D], "W1")
        Wp1 = wrk.tile([L, D], f16, tag="Wp1")
        nc.vector.tensor_scalar(Wp1[:], W1[:], cL_col[1][:], None, OP.mult)
        pmt1 = psA.tile([D, D], f32, tag="A")
        nc.tensor.matmul(pmt1[:], KV[1][:, 0:D], Wp1[:], start=True, stop=True)
        a1 = wrk.tile([D, D], f32, tag="a1")
        nc.vector.scalar_tensor_tensor(
            a1[:], MS16[:, 0:D], AL_col[:], pmt1[:], OP.mult, OP.add
        )
        MTf = wrk.tile([D, D], f16, tag="MTf")
        nc.vector.scalar_tensor_tensor(
            MTf[:], MS16[:, D : 2 * D], bL_col[:], a1[:], OP.mult, OP.add
        )

        # ---- head ----
        pmm = psT.tile([D, 1], f32, tag="t")
        nc.tensor.matmul(pmm[:], MTf[:], q16[:], start=True, stop=True)
        m32 = tny.tile([D, 1], f32, tag="m32")
        nc.vector.tensor_copy(m32[:], pmm[:])
        m16 = tny.tile([D, 1], f16, tag="m16")
        nc.scalar.copy(m16[:], pmm[:])
        dfm = tny.tile([D, 1], f32, tag="dfm")
        nc.vector.tensor_sub(dfm[:], fT[:, T - 1 : T], m32[:])

        pg = psT.tile([D, 1], f32, tag="t")
        nc.tensor.matmul(pg[:], b16[:, C_WFA : C_WFA + D], fT16[:, T - 1 : T], start=True, stop=False)
        nc.tensor.matmul(pg[:], b16[:, C_WFB : C_WFB + D], m16[:], start=False, stop=True)
        eg = tny.tile([D, 1], f32, tag="eg")
        nc.scalar.activation(eg[:], pg[:], AF.Exp, scale=-1.0, bias=b32[:, B_NBF : B_NBF + 1])
        dg = tny.tile([D, 1], f32, tag="dg")
        nc.vector.tensor_scalar(dg[:], eg[:], 1.0, None, OP.add)
        g_col = tny.tile([D, 1], f32, tag="g_col")
        nc.vector.reciprocal(g_col[:], dg[:])
        fused = tny.tile([D, 1], f16, tag="fused")
        nc.vector.scalar_tensor_tensor(
            fused[:], dfm[:], g_col[:], m32[:], OP.mult, OP.add
        )

        py = psT.tile([1, D], f32, tag="t")
        nc.tensor.matmul(py[:], fused[:], b16[:, C_W1 : C_W1 + D], start=True, stop=True)
        yb = tny.tile([1, D], f32, tag="yb")
        nc.vector.tensor_add(yb[:], py[:], rws[:, R_B1 : R_B1 + D])

        mu = tny.tile([1, 1], f32, tag="mu")
        nc.vector.tensor_reduce(mu[:], yb[:], mybir.AxisListType.X, OP.add)
        nc.vector.tensor_scalar(mu[:], mu[:], 1.0 / D, None, OP.mult)
        xc = tny.tile([1, D], f32, tag="xc")
        nc.vector.tensor_scalar(xc[:], yb[:], mu[:], None, OP.subtract)
        sq = tny.tile([1, D], f32, tag="sq")
        nc.vector.tensor_mul(sq[:], xc[:], xc[:])
        var = tny.tile([1, 1], f32, tag="var")
        nc.vector.tensor_reduce(var[:], sq[:], mybir.AxisListType.X, OP.add)
        vpe = tny.tile([1, 1], f32, tag="vpe")
        nc.vector.tensor_scalar(vpe[:], var[:], 1.0 / D, LN_EPS, OP.mult, OP.add)
        rstd = tny.tile([1, 1], f32, tag="rstd")
        nc.gpsimd.tensor_tensor(rstd[:], vpe[:], mhalf[:], OP.pow)

        xg = tny.tile([1, D], f32, tag="xg")
        nc.vector.tensor_mul(xg[:], xc[:], rws[:, R_G1 : R_G1 + D])
        xx = tny.tile([1, D], f32, tag="xx")
        nc.vector.scalar_tensor_tensor(
            xx[:], xg[:], rstd[:], rws[:, R_BE1 : R_BE1 + D], OP.mult, OP.add
        )
        # gelu(x) = x * sigmoid(GC2*(x + 0.044715 x^3))
        s1 = tny.tile([1, D], f32, tag="s1")
        nc.vector.tensor_mul(s1[:], xx[:], xx[:])
        s2 = tny.tile([1, D], f32, tag="s2")
        nc.vector.tensor_scalar(s2[:], s1[:], 0.044715, 1.0, OP.mult, OP.add)
        s3 = tny.tile([1, D], f32, tag="s3")
        nc.vector.tensor_mul(s3[:], s2[:], xx[:])
        eh = tny.tile([1, D], f32, tag="eh")
        nc.scalar.activation(eh[:], s3[:], AF.Exp, scale=-GC2)
        dh = tny.tile([1, D], f32, tag="dh")
        nc.vector.tensor_scalar(dh[:], eh[:], 1.0, None, OP.add)
        rh = tny.tile([1, D], f32, tag="rh")
        nc.vector.reciprocal(rh[:], dh[:])
        h16 = tny.tile([1, D], f16, tag="h16")
        nc.vector.tensor_mul(h16[:], xx[:], rh[:])

        phc = psT.tile([D, 1], f32, tag="t")
        nc.tensor.matmul(phc[:], h16[:], one11h[:], start=True, stop=True)
        h_col = p2s(phc[:], [D, 1], "h_col", pool=tny)

        po1 = psA.tile([1, 512], f32, tag="A")
        nc.tensor.matmul(po1[:], h_col[:], b16[:, C_W2 : C_W2 + 512], start=True, stop=True)
        po2 = psA.tile([1, OUTN - 512], f32, tag="A")
        nc.tensor.matmul(po2[:], h_col[:], b16[:, C_W2 + 512 : C_W2 + OUTN], start=True, stop=True)
        orow = cst.tile([1, OUTN], f32, tag="orow")
        nc.vector.tensor_add(orow[:, 0:512], po1[:], rws[:, R_B2 : R_B2 + 512])
        nc.vector.tensor_add(orow[:, 512:OUTN], po2[:], rws[:, R_B2 + 512 : R_B2 + OUTN])

        nc.sync.dma_start(out_d[:], orow[:])

    nc.finalize()
    return nc


def _prep_maps(inputs):
    f = np.float32
    h = np.float16
    x = np.asarray(inputs["x"], f)
    idx = np.arange(D)

    b32 = np.zeros((D, C32), f)
    b32[0:DI, B_WB : B_WB + D] = np.asarray(inputs["W_b"], f)
    wm = np.asarray(inputs["W_m"], f)
    b32[:, B_WM + 0] = wm[:, 0]
    b32[:, B_WM + 32] = wm[:, 1]
    b32[:, B_WM + 64] = wm[:, 2]
    b32[:, B_BB] = np.asarray(inputs["b_b"], f)
    b32[:, B_NBF] = -np.asarray(inputs["b_f"], f)

    b16 = np.zeros((D, C16), h)
    b16[:, C_WKV : C_WKV + D] = np.asarray(inputs["Wk"], f).astype(h)
    b16[:, C_WKV + D : C_WKV + 2 * D] = np.asarray(inputs["Wv"], f).astype(h)
    b16[:, C_WQ : C_WQ + D] = np.asarray(inputs["Wq"], f).astype(h)
    b16[:, C_W1 : C_W1 + D] = np.asarray(inputs["W1"], f).astype(h)
    b16[:, C_WFA : C_WFA + D] = np.asarray(inputs["W_f"], f)[:D].astype(h)
    b16[:, C_WFB : C_WFB + D] = np.asarray(inputs["W_f"], f)[D:].astype(h)
    b16[:, C_W2 : C_W2 + OUTN] = np.asarray(inputs["W2"], f).astype(h)
    b16[:, C_MLT : C_MLT + D] = (idx[:, None] >= idx[None, :]).astype(h)
    b16[:, C_MUT : C_MUT + D] = (idx[None, :] >= idx[:, None]).astype(h)
    b16[:, C_IDE : C_IDE + D] = np.eye(D, dtype=h)

    rws = np.zeros((1, NR), f)
    rws[0, R_B1 : R_B1 + D] = np.asarray(inputs["b1"], f)
    rws[0, R_G1 : R_G1 + D] = np.asarray(inputs["g1"], f)
    rws[0, R_BE1 : R_BE1 + D] = np.asarray(inputs["be1"], f)
    rws[0, R_B2 : R_B2 + OUTN] = np.asarray(inputs["b2"], f)

    maps = []
    for b in range(B):
        maps.append({
            "xT": np.ascontiguousarray(x[b].T),
            "b32": b32,
            "b16": b16,
            "rws": rws,
        })
    return maps


def kernel(**inputs):
    from concourse.bass_utils import run_bass_kernel_spmd

    if "nc" not in _CACHE:
        _CACHE["nc"] = _build()
    nc = _CACHE["nc"]
    maps = _prep_maps(inputs)
    res = run_bass_kernel_spmd(nc, maps, core_ids=list(range(B)))
    outs = [res.results[i]["out"].reshape(PRED_LEN, OUT_DIM) for i in range(B)]
    return np.stack(outs).astype(np.float32)


# revision 11
# speedup vs baseline: 1.6017x; 1.0717x over previous
"""Trainium2 Bass kernel for nn_ContinualForecaster (scatter_memory).

Data-parallel over batch (B=8 -> 8 NeuronCores). The T=256 recurrence on
M,S [128,128] is a chunked parallel scan (2 chunks of L=128) solving the
strictly-triangular system W = (I-N)^{-1} R via 4-level Neumann doubling,
with fp16 matmul operands (validated: rel err ~4e-3 vs 2e-2 budget).

Latency-oriented structure (v3):
- 3 ACT table loads total (Gelu, Softplus, Exp), each auto-prefetched by
  the in-order ACT queue while upstream engines work. sigmoid/log come
  from softplus identities; ln(1-0.1*sig) is a 4-term DVE polynomial.
- Both chunks' table construction and Neumann chains are emitted
  interleaved so PE/DVE/ACT pipeline across chunks.
- cL/FL state scalings folded into K (off critical path); R1 assembled
  from pre-scaled columns; gate's f-projection folded into the Exp bias.
- LN stats via accum_out; reciprocal_approx_fast for sigmoid/gelu; rstd
  via gpsimd pow(-0.5); b2 accumulated into PSUM and DMA'd straight out.
"""

import numpy as np
from contextlib import ExitStack

import sys

for _p in ("/opt/trn_rl_repo",):
    if _p not in sys.path:
        sys.path.append(_p)

B, T, DI, D = 8, 256, 64, 128
PRED_LEN, OUT_DIM = 96, 7
OUTN = PRED_LEN * OUT_DIM  # 672
L = 128
NCHUNK = T // L
NLEV = 4
LN_EPS = 1e-5
GC2 = 2.0 * 0.7978845608028654  # 2*sqrt(2/pi)

# blob16 column offsets (fp16 weights, [D, C16])
C_WKV = 0      # [Wk | Wv]  256
C_WQ = 256     # Wq         128
C_W1 = 384     # W1         128
C_WFA = 512    # W_f[:D]    128
C_WFB = 640    # W_f[D:]    128
C_W2 = 768     # W2         672
C_MLT = 1440   # lower-incl mask 128
C_MUT = 1568   # upper-incl mask 128
C_IDE = 1696   # identity   128
C16 = 1824

# blob32 column offsets (fp32, [128, C32]); Wb occupies partitions 0:64
B_WB = 0       # Wb   128
B_WM = 128     # W_m spread to cols 0/32/64 of a 65-wide block
B_BB = 193     # b_b  1
B_NBF = 194    # -b_f 1
C32 = 195

# rows32 offsets ([1, NR] fp32)
R_B1 = 0       # b1   128
R_G1 = 128     # g1   128
R_BE1 = 256    # be1  128
R_B2 = 384     # b2   672
NR = 1056

_CACHE = {}


def _build():
    import concourse.bass as bass
    import concourse.tile as tile
    from concourse import bacc, mybir

    f32 = mybir.dt.float32
    f16 = mybir.dt.float16
    AF = mybir.ActivationFunctionType
    OP = mybir.AluOpType

    nc = bacc.Bacc()

    xT_d = nc.declare_dram_parameter("xT", [DI, T], f32, isOutput=False)
    b32_d = nc.declare_dram_parameter("b32", [D, C32], f32, isOutput=False)
    b16_d = nc.declare_dram_parameter("b16", [D, C16], f16, isOutput=False)
    rws_d = nc.declare_dram_parameter("rws", [1, NR], f32, isOutput=False)
    out_d = nc.declare_dram_parameter("out", [1, OUTN], f32, isOutput=True)

    with ExitStack() as ctx:
        tc = ctx.enter_context(tile.TileContext(nc))
        cst = ctx.enter_context(tc.tile_pool(name="cst", bufs=1))
        wrk = ctx.enter_context(tc.tile_pool(name="wrk", bufs=2))
        tny = ctx.enter_context(tc.tile_pool(name="tny", bufs=2))
        psA = ctx.enter_context(
            tc.tile_pool(name="psA", bufs=2, space=bass.MemorySpace.PSUM)
        )
        psB = ctx.enter_context(
            tc.tile_pool(name="psB", bufs=4, space=bass.MemorySpace.PSUM)
        )
        psT = ctx.enter_context(
            tc.tile_pool(name="psT", bufs=2, space=bass.MemorySpace.PSUM)
        )

        # ---- tiny constants ----
        ones1 = cst.tile([1, D], f32, tag="ones1")
        nc.vector.memset(ones1[:], 1.0)
        one11 = cst.tile([1, 1], f32, tag="one11")
        nc.vector.memset(one11[:], 1.0)
        one11h = cst.tile([1, 1], f16, tag="one11h")
        nc.vector.memset(one11h[:], 1.0)
        ones_row = cst.tile([1, T], f32, tag="ones_row")
        nc.vector.memset(ones_row[:], 1.0)
        mhalf = cst.tile([1, 1], f32, tag="mhalf")
        nc.vector.memset(mhalf[:], -0.5)
        epsT = cst.tile([1, 1], f32, tag="epsT")
        nc.vector.memset(epsT[:], LN_EPS)

        # ---- input DMAs ----
        xT = cst.tile([DI, T], f32, tag="xT")
        b32 = cst.tile([D, C32], f32, tag="b32")
        b16 = cst.tile([D, C16], f16, tag="b16")
        rws = cst.tile([1, NR], f32, tag="rws")
        nc.sync.dma_start(xT[:], xT_d[:])
        nc.sync.dma_start(b32[:], b32_d[:])
        nc.gpsimd.dma_start(b16[:], b16_d[:])
        nc.gpsimd.dma_start(rws[:], rws_d[:])

        mlti = b16[:, C_MLT : C_MLT + D]
        muti = b16[:, C_MUT : C_MUT + D]
        iden = b16[:, C_IDE : C_IDE + D]

        ncopy = [0]

        def p2s(psum_ap, shape, tag, pool=wrk, dt=f16, eng=None):
            t = pool.tile(shape, dt, tag=tag)
            if eng is None:
                eng = nc.vector if ncopy[0] % 2 == 0 else nc.scalar
            if eng is nc.scalar:
                nc.scalar.copy(t[:], psum_ap)
            else:
                nc.vector.tensor_copy(t[:], psum_ap)
            ncopy[0] += 1
            return t

        def row_to_col(row_ap, n, tag, dt=f32):
            p = psT.tile([n, 1], f32, tag="t")
            nc.tensor.matmul(p[:], row_ap, one11[:], start=True, stop=True)
            return p2s(p[:], [n, 1], tag, pool=tny, dt=dt, eng=nc.vector)

        # ---- P1: backbone ----
        pf = psA.tile([D, T], f32, tag="A")
        nc.tensor.matmul(pf[:], b32[0:DI, B_WB : B_WB + D], xT[:], start=True, stop=True)
        fT = cst.tile([D, T], f32, tag="fT")
        nc.scalar.activation(fT[:], pf[:], AF.Gelu_apprx_tanh, bias=b32[:, B_BB : B_BB + 1])
        fT16 = cst.tile([D, T], f16, tag="fT16")
        nc.vector.tensor_copy(fT16[:], fT[:])

        pmt = psT.tile([65, T], f32, tag="t")
        nc.tensor.matmul(pmt[:], b32[:, B_WM : B_WM + 65], fT[:], start=True, stop=True)
        # rows 0/32/64 = theta/eta/alpha pre-acts; tanh shares the gelu
        # table, sigma(x) = 0.5 + 0.5*tanh(x/2) keeps ACT on table 0
        mtn = cst.tile([65, T], f32, tag="mtn")
        nc.scalar.activation(mtn[:], pmt[:], AF.Tanh, scale=0.5)

        # off-path early casts
        b2h = cst.tile([1, OUTN], f16, tag="b2h")
        nc.vector.tensor_copy(b2h[:], rws[:, R_B2 : R_B2 + OUTN])
        iden32 = cst.tile([D, D], f32, tag="iden32")
        nc.vector.tensor_copy(iden32[:], iden)

        # ---- P2: meta scalars ----
        th_row = cst.tile([1, T], f32, tag="th_row")
        nc.vector.tensor_scalar(th_row[:], mtn[0:1, :], 0.005, 0.005, OP.mult, OP.add)
        p_row = cst.tile([1, T], f32, tag="p_row")
        nc.vector.tensor_scalar(p_row[:], mtn[64:65, :], -0.05, 0.95, OP.mult, OP.add)
        sg1 = cst.tile([1, T], f32, tag="sg1")
        nc.vector.tensor_scalar(sg1[:], mtn[32:33, :], 0.5, 0.5, OP.mult, OP.add)
        log_et = cst.tile([1, T], f32, tag="log_et")
        nc.scalar.activation(log_et[:], sg1[:], AF.Ln)
        log_p = cst.tile([1, T], f32, tag="log_p")
        nc.scalar.activation(log_p[:], p_row[:], AF.Ln)
        nle = cst.tile([1, T + 1], f32, tag="nle")
        nc.vector.memset(nle[:, 0:1], 0.0)
        nc.vector.tensor_tensor_scan(
            nle[:, 1 : T + 1], ones_row[:], log_et[:], 0.0, OP.mult, OP.add
        )
        la_ext = cst.tile([1, T + 1], f32, tag="la_ext")
        nc.vector.memset(la_ext[:, 0:1], 0.0)
        nc.vector.tensor_tensor_scan(
            la_ext[:, 1 : T + 1], ones_row[:], log_p[:], 0.0, OP.mult, OP.add
        )

        # ---- P3: projections ----
        pk = psA.tile([D, T], f32, tag="A")
        nc.tensor.matmul(pk[:], b16[:, C_WKV : C_WKV + D], fT16[:], start=True, stop=True)
        kT = p2s(pk[:], [D, T], "kT", pool=cst)
        KV = []
        for c in range(NCHUNK):
            pkv = psA.tile([L, 2 * D], f32, tag="A")
            nc.tensor.matmul(
                pkv[:], fT16[:, c * L : (c + 1) * L], b16[:, C_WKV : C_WKV + 2 * D],
                start=True, stop=True,
            )
            KV.append(p2s(pkv[:], [L, 2 * D], f"KV{c}", pool=cst))
        pq = psT.tile([D, 1], f32, tag="t")
        nc.tensor.matmul(pq[:], b16[:, C_WQ : C_WQ + D], fT16[:, T - 1 : T], start=True, stop=True)
        q16 = p2s(pq[:], [D, 1], "q16", pool=tny, eng=nc.vector)
        # gate f-projection, folded into the sigmoid bias later
        pg1 = psT.tile([D, 1], f32, tag="t")
        nc.tensor.matmul(pg1[:], b16[:, C_WFA : C_WFA + D], fT16[:, T - 1 : T], start=True, stop=True)
        g1c = tny.tile([D, 1], f32, tag="g1c")
        nc.vector.tensor_copy(g1c[:], pg1[:])
        nb_col = tny.tile([D, 1], f32, tag="nb_col")
        nc.vector.scalar_tensor_tensor(
            nb_col[:], g1c[:], -1.0, b32[:, B_NBF : B_NBF + 1], OP.mult, OP.add
        )

        # ---- P4: per-chunk columns + R-prep scalars ----
        nle_col, la_col, th_col, negth = [], [], [], []
        for c in range(NCHUNK):
            t0 = c * L
            nle_col.append(row_to_col(nle[:, t0 + 1 : t0 + L + 1], L, f"lec{c}"))
        for c in range(NCHUNK):
            t0 = c * L
            la_col.append(row_to_col(la_ext[:, t0 + 1 : t0 + L + 1], L, f"lac{c}"))
            th_col.append(row_to_col(th_row[:, t0 : t0 + L], L, f"thc{c}"))
            nt = tny.tile([L, 1], f32, tag=f"nth{c}")
            nc.vector.tensor_scalar(nt[:], th_col[c][:], -1.0, None, OP.mult)
            negth.append(nt)

        # FL0 = exp(nle_t - nle_L)  (chunk0 S decay to chunk end)
        dfl = tny.tile([1, L], f32, tag="dfl")
        nc.vector.tensor_scalar(dfl[:], nle[:, 1 : L + 1], nle[:, L : L + 1], None, OP.subtract)
        pflp = psT.tile([L, 1], f32, tag="t")
        nc.tensor.matmul(pflp[:], dfl[:], one11[:], start=True, stop=True)
        FL_col = tny.tile([L, 1], f32, tag="FL_col")
        nc.scalar.activation(FL_col[:], pflp[:], AF.Exp, scale=-1.0)

        t1_0 = L
        # A_prev = exp(la_{t-1} - la_0) for chunk1
        dla = tny.tile([1, L], f32, tag="dla")
        nc.vector.tensor_scalar(dla[:], la_ext[:, t1_0 : t1_0 + L], la_ext[:, t1_0 : t1_0 + 1], None, OP.subtract)
        pap = psT.tile([L, 1], f32, tag="t")
        nc.tensor.matmul(pap[:], dla[:], one11[:], start=True, stop=True)
        A_prev = tny.tile([L, 1], f32, tag="A_prev")
        nc.scalar.activation(A_prev[:], pap[:], AF.Exp)
        # E_col = exp(-(nle_t - nle_0)) for chunk1
        dle = tny.tile([1, L], f32, tag="dle")
        nc.vector.tensor_scalar(dle[:], nle[:, t1_0 + 1 : t1_0 + L + 1], nle[:, t1_0 : t1_0 + 1], None, OP.subtract)
        pec = psT.tile([L, 1], f32, tag="t")
        nc.tensor.matmul(pec[:], dle[:], one11[:], start=True, stop=True)
        E_col = tny.tile([L, 1], f16, tag="E_col")
        nc.scalar.activation(E_col[:], pec[:], AF.Exp)
        # AL broadcast
        dls = tny.tile([1, 1], f32, tag="dls")
        nc.vector.tensor_scalar(dls[:], la_ext[:, t1_0 + L : t1_0 + L + 1], la_ext[:, t1_0 : t1_0 + 1], None, OP.subtract)
        als = tny.tile([1, 1], f32, tag="als")
        nc.scalar.activation(als[:], dls[:], AF.Exp)
        pal = psT.tile([D, 1], f32, tag="t")
        nc.tensor.matmul(pal[:], ones1[:], als[:], start=True, stop=True)
        AL_col = p2s(pal[:], [D, 1], "AL_col", pool=tny, dt=f32, eng=nc.vector)

        # pre-scaled R pieces (off critical path)
        thA_col = tny.tile([L, 1], f32, tag="thA_col")
        nc.vector.tensor_scalar(thA_col[:], A_prev[:], negth[1][:], None, OP.mult)
        thV1 = wrk.tile([L, D], f32, tag="thV1")
        nc.vector.tensor_scalar(thV1[:], KV[1][:, D : 2 * D], th_col[1][:], None, OP.mult)
        R0 = wrk.tile([L, D], f16, tag="R0")
        nc.vector.tensor_scalar(R0[:], KV[0][:, D : 2 * D], th_col[0][:], None, OP.mult)

        # ---- P5: tables + N, both chunks interleaved per stage ----
        pleb, mmin, FtTm, plab, dneg, Gtm, gsh = [None] * 2, [None] * 2, [None] * 2, [None] * 2, [None] * 2, [None] * 2, [None] * 2
        pct, ppsi, psi16, cL_col, n32 = [None] * 2, [None] * 2, [None] * 2, [None] * 2, [None] * 2
        for c in range(NCHUNK):
            t0 = c * L
            p = psB.tile([D, L], f32, tag="B")
            nc.tensor.matmul(p[:], ones1[:], nle[:, t0 + 1 : t0 + L + 1], start=True, stop=True)
            pleb[c] = p
        for c in range(NCHUNK):
            m = wrk.tile([L, L], f32, tag=f"dpos{c}")
            nc.vector.tensor_scalar(m[:], pleb[c][:], nle_col[c][:], 0.0, OP.subtract, OP.max)
            mmin[c] = m
        for c in range(NCHUNK):
            ft = wrk.tile([L, L], f16, tag=f"FtT{c}")
            nc.scalar.activation(ft[:], mmin[c][:], AF.Exp, scale=-1.0)
            ftm = wrk.tile([L, L], f16, tag=f"FtTm{c}")
            nc.vector.tensor_mul(ftm[:], ft[:], mlti)
            FtTm[c] = ftm
        for c in range(NCHUNK):
            t0 = c * L
            p = psB.tile([D, L], f32, tag="B")
            nc.tensor.matmul(p[:], ones1[:], la_ext[:, t0 + 1 : t0 + L + 1], start=True, stop=True)
            plab[c] = p
        for c in range(NCHUNK):
            m = wrk.tile([L, L], f32, tag=f"dneg{c}")
            nc.vector.tensor_scalar(m[:], plab[c][:], la_col[c][:], 0.0, OP.subtract, OP.min)
            dneg[c] = m
        for c in range(NCHUNK):
            gt = wrk.tile([L, L], f16, tag=f"Gt{c}")
            nc.scalar.activation(gt[:], dneg[c][:], AF.Exp)
            gm = wrk.tile([L, L], f16, tag=f"Gtm{c}")
            nc.vector.tensor_mul(gm[:], gt[:], muti)
            Gtm[c] = gm
        for c in range(NCHUNK):
            g = wrk.tile([L, L], f16, tag=f"Gsh{c}")
            nc.vector.memset(g[:, 0:1], 0.0)
            nc.vector.tensor_copy(g[:, 1:L], Gtm[c][:, 0 : L - 1])
            gsh[c] = g
        for c in range(NCHUNK):
            t0 = c * L
            p = psB.tile([L, L], f32, tag="B")
            nc.tensor.matmul(p[:], gsh[c][:], FtTm[c][:], start=True, stop=True)
            pct[c] = p
            p2 = psB.tile([L, L], f32, tag="B")
            nc.tensor.matmul(p2[:], kT[:, t0 : t0 + L], kT[:, t0 : t0 + L], start=True, stop=True)
            ppsi[c] = p2
            pc = psT.tile([L, 1], f32, tag="t")
            nc.tensor.matmul(pc[:], FtTm[c][:], Gtm[c][:, L - 1 : L], start=True, stop=True)
            cL_col[c] = p2s(pc[:], [L, 1], f"cL{c}", pool=tny, dt=f32, eng=nc.vector)
        for c in range(NCHUNK):
            ps = wrk.tile([L, L], f16, tag=f"psi{c}")
            nc.scalar.copy(ps[:], ppsi[c][:])
            psi16[c] = ps
        for c in range(NCHUNK):
            n = wrk.tile([L, L], f32, tag=f"N32{c}")
            nc.vector.scalar_tensor_tensor(
                n[:], pct[c][:], negth[c][:], psi16[c][:], OP.mult, OP.mult
            )
            n32[c] = n

        # K pre-scaled by cL (M update) and FL (S update)
        Kcs = [None, None]
        for c in range(NCHUNK):
            kk = wrk.tile([L, D], f16, tag=f"Kcs{c}")
            nc.vector.tensor_scalar(kk[:], KV[c][:, 0:D], cL_col[c][:], None, OP.mult)
            Kcs[c] = kk
        Kfl0 = wrk.tile([L, D], f16, tag="Kfl0")
        nc.vector.tensor_scalar(Kfl0[:], KV[0][:, 0:D], FL_col[:], None, OP.mult)

        # ---- P6: chunk1 b-row pieces ----
        pb = psT.tile([1, L], f32, tag="t")
        nc.tensor.matmul(pb[:], E_col[:], Gtm[1][:], start=True, stop=True)
        b_sh = tny.tile([1, L], f32, tag="b_sh")
        nc.vector.memset(b_sh[:, 0:1], 0.0)
        nc.vector.tensor_copy(b_sh[:, 1:L], pb[:, 0 : L - 1])
        bls = tny.tile([1, 1], f32, tag="bls")
        nc.vector.tensor_copy(bls[:], pb[:, L - 1 : L])
        pbp = psT.tile([L, 1], f32, tag="t")
        nc.tensor.matmul(pbp[:], b_sh[:], one11[:], start=True, stop=True)
        b_prev = p2s(pbp[:], [L, 1], "b_prev", pool=tny, dt=f32, eng=nc.vector)
        thB_col = tny.tile([L, 1], f32, tag="thB_col")
        nc.vector.tensor_scalar(thB_col[:], b_prev[:], negth[1][:], None, OP.mult)
        pbl = psT.tile([D, 1], f32, tag="t")
        nc.tensor.matmul(pbl[:], ones1[:], bls[:], start=True, stop=True)
        bL_col = p2s(pbl[:], [D, 1], "bL_col", pool=tny, dt=f32, eng=nc.vector)

        # ---- P7: transposes + doubling, interleaved ----
        pnt = [None, None]
        X = [None, None]
        Y = [None, None]
        IV = [None, None]
        for c in range(NCHUNK):
            p = psB.tile([L, L], f32, tag="B")
            nc.tensor.transpose(p[:], n32[c][:], iden32[:])
            pnt[c] = p
        for c in range(NCHUNK):
            x = wrk.tile([L, L], f16, tag=f"X{c}")
            nc.vector.tensor_copy(x[:], n32[c][:])
            X[c] = x
            y = wrk.tile([L, L], f16, tag=f"Y{c}")
            nc.scalar.copy(y[:], pnt[c][:])
            Y[c] = y
            iv = wrk.tile([L, L], f16, tag=f"IV{c}")
            nc.vector.tensor_add(iv[:], pnt[c][:], iden)
            IV[c] = iv
        for lev in range(1, NLEV + 1):
            px2 = [None, None]
            py2 = [None, None]
            for c in range(NCHUNK):
                pxt = psB.tile([L, L], f32, tag="B", name=f"px2_{c}")
                nc.tensor.matmul(pxt[:], Y[c][:], X[c][:], start=True, stop=True)
                px2[c] = pxt
            X2 = [None, None]
            for c in range(NCHUNK):
                X2[c] = p2s(px2[c][:], [L, L], f"X{c}")
            if lev < NLEV:
                for c in range(NCHUNK):
                    pyt = psB.tile([L, L], f32, tag="B", name=f"py2_{c}")
                    nc.tensor.matmul(pyt[:], X[c][:], Y[c][:], start=True, stop=True)
                    py2[c] = pyt
                for c in range(NCHUNK):
                    Y[c] = p2s(py2[c][:], [L, L], f"Y{c}")
            for c in range(NCHUNK):
                X[c] = X2[c]
            for c in range(NCHUNK):
                piu = psB.tile([L, L], f32, tag="B")
                nc.tensor.matmul(piu[:], X[c][:], IV[c][:], start=True, stop=True)
                iv2 = wrk.tile([L, L], f16, tag=f"IV{c}")
                nc.vector.tensor_add(iv2[:], IV[c][:], piu[:])
                IV[c] = iv2

        # ---- P8: solves + state ----
        MS16 = cst.tile([D, 2 * D], f16, tag="MS16")
        pw0 = psA.tile([L, D], f32, tag="A")
        nc.tensor.matmul(pw0[:], IV[0][:], R0[:], start=True, stop=True)
        W0 = p2s(pw0[:], [L, D], "W0", eng=nc.vector)
        pmt0 = psA.tile([D, D], f32, tag="A")
        nc.tensor.matmul(pmt0[:], Kcs[0][:], W0[:], start=True, stop=True)
        nc.vector.tensor_copy(MS16[:, 0:D], pmt0[:])
        pst0 = psA.tile([D, D], f32, tag="A")
        nc.tensor.matmul(pst0[:], Kfl0[:], W0[:], start=True, stop=True)
        nc.scalar.copy(MS16[:, D : 2 * D], pst0[:])

        pzm = psA.tile([L, D], f32, tag="A")
        nc.tensor.matmul(pzm[:], kT[:, t1_0 : t1_0 + L], MS16[:, 0:D], start=True, stop=True)
        r1a = wrk.tile([L, D], f32, tag="r1a")
        nc.vector.scalar_tensor_tensor(
            r1a[:], pzm[:], thA_col[:], thV1[:], OP.mult, OP.add
        )
        pzs = psA.tile([L, D], f32, tag="A")
        nc.tensor.matmul(pzs[:], kT[:, t1_0 : t1_0 + L], MS16[:, D : 2 * D], start=True, stop=True)
        R1 = wrk.tile([L, D], f16, tag="R1")
        nc.vector.scalar_tensor_tensor(
            R1[:], pzs[:], thB_col[:], r1a[:], OP.mult, OP.add
        )

        pw1 = psA.tile([L, D], f32, tag="A")
        nc.tensor.matmul(pw1[:], IV[1][:], R1[:], start=True, stop=True)
        W1 = p2s(pw1[:], [L, D], "W1", eng=nc.vector)
        pmt1 = psA.tile([D, D], f32, tag="A")
        nc.tensor.matmul(pmt1[:], Kcs[1][:], W1[:], start=True, stop=True)
        a1 = wrk.tile([D, D], f32, tag="a1")
        nc.vector.scalar_tensor_tensor(
            a1[:], MS16[:, 0:D], AL_col[:], pmt1[:], OP.mult, OP.add
        )
        MTf = wrk.tile([D, D], f16, tag="MTf")
        nc.vector.scalar_tensor_tensor(
            MTf[:], MS16[:, D : 2 * D], bL_col[:], a1[:], OP.mult, OP.add
        )

        # ---- P9: head ----
        pmm = psT.tile([D, 1], f32, tag="t")
        nc.tensor.matmul(pmm[:], MTf[:], q16[:], start=True, stop=True)
        m32 = tny.tile([D, 1], f32, tag="m32")
        nc.vector.tensor_copy(m32[:], pmm[:])
        m16 = tny.tile([D, 1], f16, tag="m16")
        nc.vector.tensor_copy(m16[:], pmm[:])
        dfm = tny.tile([D, 1], f32, tag="dfm")
        nc.vector.tensor_sub(dfm[:], fT[:, T - 1 : T], m32[:])

        pgg = psT.tile([D, 1], f32, tag="t")
        nc.tensor.matmul(pgg[:], b16[:, C_WFB : C_WFB + D], m16[:], start=True, stop=True)
        eg = tny.tile([D, 1], f32, tag="eg")
        nc.scalar.activation(eg[:], pgg[:], AF.Exp, scale=-1.0, bias=nb_col[:])
        dg = tny.tile([D, 1], f32, tag="dg")
        nc.vector.tensor_scalar(dg[:], eg[:], 1.0, None, OP.add)
        gr = tny.tile([D, 1], f32, tag="gr")
        nc.vector.reciprocal_approx_fast(gr[:], dg[:])
        fused = tny.tile([D, 1], f16, tag="fused")
        nc.vector.scalar_tensor_tensor(
            fused[:], dfm[:], gr[:], m32[:], OP.mult, OP.add
        )

        py = psT.tile([1, D], f32, tag="t")
        nc.tensor.matmul(py[:], fused[:], b16[:, C_W1 : C_W1 + D], start=True, stop=True)
        yb = tny.tile([1, D], f32, tag="yb")
        musum = tny.tile([1, 1], f32, tag="musum")
        nc.vector.scalar_tensor_tensor(
            yb[:], py[:], 1.0, rws[:, R_B1 : R_B1 + D], OP.mult, OP.add,
            accum_out=musum[:],
        )
        y2scr = tny.tile([1, D], f32, tag="y2scr")
        y2sum = tny.tile([1, 1], f32, tag="y2sum")
        nc.vector.scalar_tensor_tensor(
            y2scr[:], yb[:], 0.0, yb[:], OP.add, OP.mult, accum_out=y2sum[:]
        )
        mu = tny.tile([1, 1], f32, tag="mu")
        nc.vector.tensor_scalar(mu[:], musum[:], 1.0 / D, None, OP.mult)
        mu2e = tny.tile([1, 1], f32, tag="mu2e")
        nc.vector.scalar_tensor_tensor(mu2e[:], mu[:], mu[:], epsT[:], OP.mult, OP.subtract)
        vpe = tny.tile([1, 1], f32, tag="vpe")
        nc.vector.scalar_tensor_tensor(vpe[:], y2sum[:], 1.0 / D, mu2e[:], OP.mult, OP.subtract)
        xg = tny.tile([1, D], f32, tag="xg")
        nc.vector.scalar_tensor_tensor(
            xg[:], yb[:], mu[:], rws[:, R_G1 : R_G1 + D], OP.subtract, OP.mult
        )
        rstd = tny.tile([1, 1], f32, tag="rstd")
        nc.gpsimd.tensor_tensor(rstd[:], vpe[:], mhalf[:], OP.pow)
        xx = tny.tile([1, D], f32, tag="xx")
        nc.vector.scalar_tensor_tensor(
            xx[:], xg[:], rstd[:], rws[:, R_BE1 : R_BE1 + D], OP.mult, OP.add
        )
        s1 = tny.tile([1, D], f32, tag="s1")
        nc.vector.tensor_mul(s1[:], xx[:], xx[:])
        s2 = tny.tile([1, D], f32, tag="s2")
        nc.vector.tensor_scalar(s2[:], s1[:], 0.044715, 1.0, OP.mult, OP.add)
        s3 = tny.tile([1, D], f32, tag="s3")
        nc.vector.tensor_mul(s3[:], s2[:], xx[:])
        eh = tny.tile([1, D], f32, tag="eh")
        nc.scalar.activation(eh[:], s3[:], AF.Exp, scale=-GC2)
        dh = tny.tile([1, D], f32, tag="dh")
        nc.vector.tensor_scalar(dh[:], eh[:], 1.0, None, OP.add)
        rh = tny.tile([1, D], f32, tag="rh")
        nc.vector.reciprocal_approx_fast(rh[:], dh[:])
        h16 = tny.tile([1, D], f16, tag="h16")
        nc.vector.tensor_mul(h16[:], xx[:], rh[:])

        phc = psT.tile([D, 1], f32, tag="t")
        nc.tensor.matmul(phc[:], h16[:], one11h[:], start=True, stop=True)
        h_col = p2s(phc[:], [D, 1], "h_col", pool=tny, eng=nc.vector)

        po1 = psA.tile([1, 512], f32, tag="A")
        nc.tensor.matmul(po1[:], h_col[:], b16[:, C_W2 : C_W2 + 512], start=True, stop=False)
        nc.tensor.matmul(po1[:], one11h[:], b2h[:, 0:512], start=False, stop=True)
        po2 = psA.tile([1, OUTN - 512], f32, tag="A")
        nc.tensor.matmul(po2[:], h_col[:], b16[:, C_W2 + 512 : C_W2 + OUTN], start=True, stop=False)
        nc.tensor.matmul(po2[:], one11h[:], b2h[:, 512:OUTN], start=False, stop=True)
        orow = cst.tile([1, OUTN], f32, tag="orow")
        nc.vector.tensor_copy(orow[:, 0:512], po1[:])
        nc.vector.tensor_copy(orow[:, 512:OUTN], po2[:])
        nc.sync.dma_start(out_d[:], orow[:])

    nc.finalize()
    return nc


def _prep_maps(inputs):
    f = np.float32
    h = np.float16
    x = np.asarray(inputs["x"], f)
    idx = np.arange(D)

    b32 = np.zeros((D, C32), f)
    b32[0:DI, B_WB : B_WB + D] = np.asarray(inputs["W_b"], f)
    wm = np.asarray(inputs["W_m"], f)
    b32[:, B_WM + 0] = wm[:, 0]
    b32[:, B_WM + 32] = wm[:, 1]
    b32[:, B_WM + 64] = wm[:, 2]
    b32[:, B_BB] = np.asarray(inputs["b_b"], f)
    b32[:, B_NBF] = -np.asarray(inputs["b_f"], f)

    b16 = np.zeros((D, C16), h)
    b16[:, C_WKV : C_WKV + D] = np.asarray(inputs["Wk"], f).astype(h)
    b16[:, C_WKV + D : C_WKV + 2 * D] = np.asarray(inputs["Wv"], f).astype(h)
    b16[:, C_WQ : C_WQ + D] = np.asarray(inputs["Wq"], f).astype(h)
    b16[:, C_W1 : C_W1 + D] = np.asarray(inputs["W1"], f).astype(h)
    b16[:, C_WFA : C_WFA + D] = np.asarray(inputs["W_f"], f)[:D].astype(h)
    b16[:, C_WFB : C_WFB + D] = np.asarray(inputs["W_f"], f)[D:].astype(h)
    b16[:, C_W2 : C_W2 + OUTN] = np.asarray(inputs["W2"], f).astype(h)
    b16[:, C_MLT : C_MLT + D] = (idx[:, None] >= idx[None, :]).astype(h)
    b16[:, C_MUT : C_MUT + D] = (idx[None, :] >= idx[:, None]).astype(h)
    b16[:, C_IDE : C_IDE + D] = np.eye(D, dtype=h)

    rws = np.zeros((1, NR), f)
    rws[0, R_B1 : R_B1 + D] = np.asarray(inputs["b1"], f)
    rws[0, R_G1 : R_G1 + D] = np.asarray(inputs["g1"], f)
    rws[0, R_BE1 : R_BE1 + D] = np.asarray(inputs["be1"], f)
    rws[0, R_B2 : R_B2 + OUTN] = np.asarray(inputs["b2"], f)

    maps = []
    for b in range(B):
        maps.append({
            "xT": np.ascontiguousarray(x[b].T),
            "b32": b32,
            "b16": b16,
            "rws": rws,
        })
    return maps


def kernel(**inputs):
    from concourse.bass_utils import run_bass_kernel_spmd

    if "nc" not in _CACHE:
        _CACHE["nc"] = _build()
    nc = _CACHE["nc"]
    maps = _prep_maps(inputs)
    res = run_bass_kernel_spmd(nc, maps, core_ids=list(range(B)))
    outs = [res.results[i]["out"].reshape(PRED_LEN, OUT_DIM) for i in range(B)]
    return np.stack(outs).astype(np.float32)
